# revision 4
# baseline (speedup 1.0000x reference)
"""RNN kernel v5: v4 (phase A interleaved into phase B) + ACT writes fp16 h
directly into the output staging tile, which doubles as the h source for the
next step's matmuls — no DVE cast on the recurrence critical path.
"""

import os
import sys
import types

import numpy as np

import concourse.mybir as mybir

try:
    from antenv.axon_hooks import get_axon_ntff_profile_hook  # noqa: F401
except ImportError:
    from trn_agent_boot.trn_boot import _ntff_profile_via_ctypes

    _hook = _ntff_profile_via_ctypes("/opt/axon/libaxon_pjrt.so")
    _mod = types.ModuleType("antenv.axon_hooks")
    _mod.get_axon_ntff_profile_hook = lambda: _hook
    sys.modules["antenv.axon_hooks"] = _mod

from concourse import bacc, bass, tile
from concourse.bass_utils import run_bass_kernel_spmd

FP32 = mybir.dt.float32
F16 = mybir.dt.float16
AF = mybir.ActivationFunctionType

T, B, D = 256, 256, 1024
NCORES = 8
BL = B // NCORES          # 32
HB = 8                    # timesteps per block
NHB = T // HB             # 32 blocks
COLS = HB * BL            # 256

_cached = {}


def _build():
    if "nc" in _cached:
        return _cached["nc"]

    nc = bacc.Bacc(None, target_bir_lowering=False, debug=True)
    wx_in = nc.dram_tensor("wx", [128, 64 * 128], F16, kind="ExternalInput")
    wh_in = nc.dram_tensor("wh", [128, 64 * 128], F16, kind="ExternalInput")
    bb_in = nc.dram_tensor("bias_bc", [128, 2048], FP32, kind="ExternalInput")
    xt_in = nc.dram_tensor("xt", [128, 8 * T * BL], F16, kind="ExternalInput")
    out_dr = nc.dram_tensor("out", [128, 8 * T * BL], F16, kind="ExternalOutput")

    with tile.TileContext(nc) as tc:
        with tc.tile_pool(name="sbuf", bufs=1) as pool, \
             tc.tile_pool(name="psum", bufs=1, space=bass.MemorySpace.PSUM) as ppool:
            wx_sb = pool.tile([128, 64 * 128], F16)
            wh_sb = pool.tile([128, 64 * 128], F16)
            bb_sb = pool.tile([128, 2048], FP32)
            nc.gpsimd.dma_start(out=wx_sb[:], in_=wx_in[:])
            nc.gpsimd.dma_start(out=wh_sb[:], in_=wh_in[:])
            nc.gpsimd.dma_start(out=bb_sb[:], in_=bb_in[:])

            xb0 = pool.tile([128, 8 * COLS], F16)
            xb1 = pool.tile([128, 8 * COLS], F16)
            xbufs = [xb0, xb1]

            obufs = [pool.tile([128, HB * 256], F16, name=f"ob{s}") for s in range(3)]

            hz = pool.tile([128, 256], F16)
            nc.vector.memset(hz[:], 0.0)

            ps = ppool.tile([128, 4096], FP32)

            def x_dma(hb):
                xb = xbufs[hb % 2]
                for kg in range(8):
                    nc.gpsimd.dma_start(
                        out=xb[:, kg * COLS:(kg + 1) * COLS],
                        in_=xt_in[:, kg * T * BL + hb * COLS: kg * T * BL + (hb + 1) * COLS],
                    )

            def a_mms(hb):
                base = (hb % 2) * 2048
                xb = xbufs[hb % 2]
                thunks = []
                for jg in range(8):
                    reg = ps[:, base + jg * COLS: base + (jg + 1) * COLS]
                    for kg in range(8):
                        def wx_mm(reg=reg, jg=jg, kg=kg):
                            nc.tensor.matmul(
                                reg,
                                wx_sb[:, (kg * 8 + jg) * 128:(kg * 8 + jg + 1) * 128],
                                xb[:, kg * COLS:(kg + 1) * COLS],
                                start=False, stop=(kg == 7),
                                skip_group_check=True,
                            )
                        thunks.append(wx_mm)
                return thunks

            x_dma(0)
            x_dma(1)
            nc.vector.tensor_copy(out=ps[:, 0:2048], in_=bb_sb[:])
            for th in a_mms(0):
                th()

            for hb in range(NHB):
                base = (hb % 2) * 2048
                ps3 = ps[:, base:base + 2048].rearrange("p (j c) -> p j c", j=8)
                if hb + 2 < NHB:
                    x_dma(hb + 2)
                if hb + 1 < NHB:
                    b2 = ((hb + 1) % 2) * 2048
                    nc.vector.tensor_copy(out=ps[:, b2:b2 + 2048], in_=bb_sb[:])
                amms = a_mms(hb + 1) if hb + 1 < NHB else []
                ai = 0
                ob = obufs[hb % 3]
                for lt in range(HB):
                    if hb == 0 and lt == 0:
                        cur = hz[:]
                    elif lt == 0:
                        cur = obufs[(hb - 1) % 3][:, (HB - 1) * 256: HB * 256]
                    else:
                        cur = ob[:, (lt - 1) * 256: lt * 256]
                    for half in range(2):
                        for kg in range(8):
                            for jg in range(half * 4, half * 4 + 4):
                                nc.tensor.matmul(
                                    ps[:, base + jg * COLS + lt * BL:
                                       base + jg * COLS + (lt + 1) * BL],
                                    wh_sb[:, (kg * 8 + jg) * 128:(kg * 8 + jg + 1) * 128],
                                    cur[:, kg * BL:(kg + 1) * BL],
                                    start=False,
                                    stop=(kg == 7),
                                    skip_group_check=True,
                                )
                        for _ in range(4):
                            if ai < len(amms):
                                amms[ai]()
                                ai += 1
                        ob2 = ob[:, lt * 256:(lt + 1) * 256].rearrange(
                            "p (j c) -> p j c", j=8)
                        nc.scalar.activation(
                            out=ob2[:, half * 4:(half + 1) * 4, :],
                            in_=ps3[:, half * 4:(half + 1) * 4, lt * BL:(lt + 1) * BL],
                            func=AF.Tanh,
                            bias=0.0,
                            scale=1.0,
                        )
                while ai < len(amms):
                    amms[ai]()
                    ai += 1
                nc.sync.dma_start(
                    out=out_dr[:, hb * HB * 256:(hb + 1) * HB * 256],
                    in_=ob[:, :],
                )

    nc.compile()
    _cached["nc"] = nc
    return nc


def kernel(x: np.ndarray, W: np.ndarray, b: np.ndarray) -> np.ndarray:
    nc = _build()

    Wx = W[:, :D]
    Wh = W[:, D:]
    wx_np = np.ascontiguousarray(
        Wx.reshape(8, 128, 8, 128).transpose(3, 2, 0, 1).reshape(128, 64 * 128)
    ).astype(np.float16)
    wh_np = np.ascontiguousarray(
        Wh.reshape(8, 128, 8, 128).transpose(3, 2, 0, 1).reshape(128, 64 * 128)
    ).astype(np.float16)
    # bias_bc[p, jg*256+c] = b[jg*128+p], fp32
    bb_np = np.ascontiguousarray(
        np.broadcast_to(
            b.reshape(8, 128).T[:, :, None], (128, 8, COLS)
        ).reshape(128, 2048)
    ).astype(np.float32)

    ins = []
    for c in range(NCORES):
        xc = x[:, c * BL:(c + 1) * BL, :]
        xT = xc.reshape(T * BL, D).T
        xt_np = np.ascontiguousarray(
            xT.reshape(8, 128, T * BL).transpose(1, 0, 2).reshape(128, 8 * T * BL)
        ).astype(np.float16)
        ins.append({"wx": wx_np, "wh": wh_np, "bias_bc": bb_np, "xt": xt_np})

    trace = bool(os.environ.get("BASS_KERNEL_TRACE"))
    res = run_bass_kernel_spmd(nc, ins, list(range(NCORES)), trace=trace)
    if trace:
        _cached["exec_time_ns"] = res.exec_time_ns

    out = np.empty((B, T, D), np.float32)
    for c in range(NCORES):
        oc = np.asarray(res.results[c]["out"])
        # oc[p, hb*2048 + lt*256 + jg*32 + b] = h_{hb*8+lt}[jg*128+p, b]
        out[c * BL:(c + 1) * BL] = (
            oc.reshape(128, NHB, HB, 8, BL)
            .transpose(4, 1, 2, 3, 0)
            .reshape(BL, T, D)
            .astype(np.float32)
        )
    return out


if __name__ == "__main__":
    rng = np.random.default_rng(0)
    x = rng.standard_normal((T, B, D)).astype(np.float32)
    W = ((rng.uniform(-1, 1, (D, 2 * D))) / np.sqrt(2 * D)).astype(np.float32)
    b = ((rng.uniform(-1, 1, D)) / np.sqrt(2 * D)).astype(np.float32)
    got = kernel(x, W, b)
    if "exec_time_ns" in _cached:
        print("HW exec time:", _cached["exec_time_ns"], "ns")
    Wx, Wh = W[:, :D], W[:, D:]
    h = np.zeros((B, D), np.float32)
    ref = np.empty((B, T, D), np.float32)
    for t in range(T):
        h = np.tanh(x[t] @ Wx.T + h @ Wh.T + b)
        ref[:, t, :] = h
    err = np.abs(got - ref).max() / np.abs(ref).max()
    print("self-check rel err:", err)
